# revision 2
# baseline (speedup 1.0000x reference)
"""Trainium2 Bass kernel for nn_Decoder_48859547959519.

Autoregressive LSTM decoder: 512 sequential steps, batch 8, hidden 256,
feedback y_t = fc(h_{t+1}) -> x_{t+1}.

Strategy (data parallel, 1 batch element per NeuronCore, 8 cores):
  * Algebraic fusion: x_{t+1} = W_fc h_{t+1} + b_fc  =>  for t >= 1
        gates_t = (W_ih W_fc + W_hh) h_t + (W_ih b_fc + b) = W_eff h_t + b_eff
    so the per-step critical path is a single 256->1024 matvec + LSTM cell.
    Outputs are reconstructed at the end with one batched matmul over the
    stored hidden-state history.
  * Step 0 (x_0 = 0) is peeled on the host (pure input preprocessing).
  * Weights stationary on the PE (lhsT = W_eff^T tiles, bf16, FWL), rhs = h.
    Gates land transposed: [gate-dim on partitions, 1 col per 128-chunk],
    the layout ACT/DVE need for the elementwise cell update.
  * b_eff is preloaded into PSUM with one identity-matmul (lhsT = packed
    biases, rhs = I_8), so the W-matmuls accumulate on top of it.
  * All transcendentals are Sigmoid (one ACT table set):
    tanh(x) = 2*sigmoid(2x) - 1, with the 2x folded into the g-gate rows
    of W_eff / b_eff at weight-prep time.
"""

import numpy as np

SEQ_LEN = 512
IN_DIM = 23
HID = 256
FEAT = 128
BATCH = 8
NCHUNK = 8  # 4*HID / 128
# chunk column order in PSUM: [i0 i1 f0 f1 o0 o1 g0 g1]
CHUNK_ROWS = [0, 128, 256, 384, 768, 896, 512, 640]
# Each For_i back-edge costs ~5.3us (full multi-engine barrier + branch),
# measured from the ntff trace. 256 halves the back-edge count vs 128;
# the 11.5k-inst body is still well under the full-unroll size that
# showed fetch stalls.
UNROLL = 256

_CACHE = {}


def _sigmoid(x):
    return 1.0 / (1.0 + np.exp(-x))


def _host_prep(feature, W_ih, W_hh, b_ih, b_hh, W_fc, b_fc, W_hfc, b_hfc):
    """Fuse the feedback path, peel step 0, pack device tensors."""
    f32 = np.float32
    W_ih = np.asarray(W_ih, f32)
    W_hh = np.asarray(W_hh, f32)
    W_fc = np.asarray(W_fc, f32)
    b = np.asarray(b_ih, f32) + np.asarray(b_hh, f32)

    W_eff = (W_ih @ W_fc + W_hh).astype(f32)          # [1024, 256]
    b_eff = (W_ih @ np.asarray(b_fc, f32) + b).astype(f32)  # [1024]
    # fold tanh(g) = 2*sigmoid(2g) - 1 into the g rows (ACT ops are ~290ns,
    # a DVE tensor_scalar is ~170ns, so one sigmoid over all gates wins)
    W_eff = W_eff.copy()
    b_eff = b_eff.copy()
    W_eff[512:768] *= 2.0
    b_eff[512:768] *= 2.0

    # step 0 on host (x_0 = 0): h0 from feature, c0 = 0
    feats = np.asarray(feature, f32)                  # [B, FEAT]
    h0 = feats @ np.asarray(W_hfc, f32).T + np.asarray(b_hfc, f32)  # [B, HID]
    g0 = h0 @ W_hh.T + b                              # [B, 1024]
    i_g, f_g, g_g, o_g = np.split(g0, 4, axis=1)
    c1 = _sigmoid(i_g) * np.tanh(g_g)                 # [B, HID]
    h1 = _sigmoid(o_g) * np.tanh(c1)                  # [B, HID]

    # pack weight tiles: wt[p, k*1024 + m*128 + j] = W_eff[row(m)+j, k*128+p]
    wt = np.empty((128, 2048), np.float32)
    for k in range(2):
        for m in range(NCHUNK):
            blk = W_eff[CHUNK_ROWS[m]:CHUNK_ROWS[m] + 128,
                        k * 128:(k + 1) * 128]        # [j, p]
            wt[:, k * 1024 + m * 128:k * 1024 + (m + 1) * 128] = blk.T
    bpack = np.stack([b_eff[r:r + 128] for r in CHUNK_ROWS])  # [8, 128]
    eye8 = np.eye(8, dtype=np.float32)

    # fc weights for the output stage: wfc[p, k*23+d] = W_fc[d, k*128+p]
    wfc = np.empty((128, 2 * IN_DIM), np.float32)
    for k in range(2):
        wfc[:, k * IN_DIM:(k + 1) * IN_DIM] = W_fc[:, k * 128:(k + 1) * 128].T
    bfc = np.asarray(b_fc, f32).reshape(IN_DIM, 1)

    import ml_dtypes
    bf16 = ml_dtypes.bfloat16
    per_core = []
    for bb in range(BATCH):
        per_core.append({
            "wt": wt.astype(bf16),
            "bpack": bpack.astype(bf16),
            "eye8": eye8.astype(bf16),
            "wfc": wfc.astype(bf16),
            "bfc": bfc,
            "h1": np.stack([h1[bb, 0:128], h1[bb, 128:256]], 1).astype(bf16),
            "c1": np.stack([c1[bb, 0:128], c1[bb, 128:256]], 1).astype(f32),
        })
    return per_core


def build_program(T=SEQ_LEN, unroll=UNROLL):
    """Emit the Bass/Tile program.

    unroll == T: fully unrolled straight-line loop (no back-edges, static
    history APs, one fewer step since no uniformity padding is needed).
    Otherwise: For_i dynamic loop over T steps in chunks of `unroll`.
    """
    import concourse.bacc as bacc
    import concourse.bass as bass
    import concourse.mybir as mybir
    import concourse.tile as tile

    f32 = mybir.dt.float32
    bf16 = mybir.dt.bfloat16
    SIG = mybir.ActivationFunctionType.Sigmoid
    TANH = mybir.ActivationFunctionType.Tanh
    IDT = mybir.ActivationFunctionType.Identity
    ALU = mybir.AluOpType

    assert T % unroll == 0
    nc = bacc.Bacc("TRN2", target_bir_lowering=False, debug=False)

    # DRAM I/O
    wt_d = nc.dram_tensor("wt", [128, 2048], bf16, kind="ExternalInput")
    bp_d = nc.dram_tensor("bpack", [8, 128], bf16, kind="ExternalInput")
    i8_d = nc.dram_tensor("eye8", [8, 8], bf16, kind="ExternalInput")
    wfc_d = nc.dram_tensor("wfc", [128, 2 * IN_DIM], bf16, kind="ExternalInput")
    bfc_d = nc.dram_tensor("bfc", [IN_DIM, 1], f32, kind="ExternalInput")
    h1_d = nc.dram_tensor("h1", [128, 2], bf16, kind="ExternalInput")
    c1_d = nc.dram_tensor("c1", [128, 2], f32, kind="ExternalInput")
    yt_d = nc.dram_tensor("yt", [IN_DIM, T], f32, kind="ExternalOutput")

    # persistent SBUF state
    wt_s = nc.alloc_sbuf_tensor("wt_s", [128, 2048], bf16)
    bp_s = nc.alloc_sbuf_tensor("bp_s", [8, 128], bf16)
    i8_s = nc.alloc_sbuf_tensor("i8_s", [8, 8], bf16)
    wfc_s = nc.alloc_sbuf_tensor("wfc_s", [128, 2 * IN_DIM], bf16)
    bfc_s = nc.alloc_sbuf_tensor("bfc_s", [IN_DIM, 1], f32)
    h_s = nc.alloc_sbuf_tensor("h_s", [128, 2], bf16)
    # tgc: cols 0:2 = per-step tanh(g) scratch, cols 2:4 = persistent c state.
    # Keeping them adjacent lets [u|v] = [s_i|s_f] * [t_g|c] run as ONE
    # tensor_tensor op.
    tgc_s = nc.alloc_sbuf_tensor("tgc_s", [128, 4], f32)
    hist = nc.alloc_sbuf_tensor("hist", [128, 2 * (T + 2)], bf16)
    ysb = nc.alloc_sbuf_tensor("ysb", [IN_DIM, T], f32)

    wt_a = wt_s.ap()
    h_a = h_s.ap()
    tgc_a = tgc_s.ap()
    hist_a = hist.ap()

    with tile.TileContext(nc) as tc:
        # upload constants + initial state
        nc.sync.dma_start(wt_a, wt_d.ap())
        nc.sync.dma_start(bp_s.ap(), bp_d.ap())
        nc.sync.dma_start(i8_s.ap(), i8_d.ap())
        nc.sync.dma_start(wfc_s.ap(), wfc_d.ap())
        nc.sync.dma_start(bfc_s.ap(), bfc_d.ap())
        nc.sync.dma_start(h_a, h1_d.ap())
        nc.sync.dma_start(hist_a[:, 0:2], h1_d.ap())
        nc.sync.dma_start(tgc_a[:, 2:4], c1_d.ap())

        with (
            tc.tile_pool(name="work", bufs=2) as wp,
            tc.tile_pool(name="gpsum", bufs=2, space="PSUM") as gp,
            tc.tile_pool(name="ypsum", bufs=1, space="PSUM") as yp,
        ):
            # Prime the sigmoid/tanh ACT table set before the loop so the
            # table-load fixpoint doesn't place a ~2.7us reload in the body.
            # Feed it from a memset scratch (not a DMA'd input) so the
            # ~2.7us table load overlaps the input DMAs.
            warm = wp.tile([1, 1], f32, tag="warm")
            nc.vector.memset(warm[:], 0.0)
            nc.scalar.activation(warm[:], warm[:], SIG)
            def step(iv):
                # ---- PE: gates = b_eff + W_eff @ h  (PSUM [128, 8]) ----
                g_ps = gp.tile([128, NCHUNK], f32, tag="gates")
                nc.tensor.matmul(g_ps[:, 0:NCHUNK], bp_s.ap(), i8_s.ap(),
                                 start=True, stop=False, skip_group_check=True)
                for k in range(2):
                    for m in range(NCHUNK):
                        nc.tensor.matmul(
                            g_ps[:, m:m + 1],
                            wt_a[:, k * 1024 + m * 128:k * 1024 + (m + 1) * 128],
                            h_a[:, k:k + 1],
                            start=False, stop=(k == 1),
                            skip_group_check=True)

                # ---- ACT: sall = sigmoid(gates); g cols hold sigmoid(2g) ----
                sall = wp.tile([128, NCHUNK], f32, tag="sall")
                nc.scalar.activation(sall[:], g_ps[:, 0:NCHUNK], SIG)

                # ---- DVE cell update (tanh(g) = 2*sig(2g)-1, folded):
                #   t'  = sig(2g) - 0.5                       (single-op TS)
                #   uv  = [s_i|s_f] * [t'|c] = [u/2 | v]      (one TT)
                #   c   = 2*(u/2) + v                         (one STT)
                nc.vector.tensor_scalar(tgc_a[:, 0:2], sall[:, 6:8], 0.5, None,
                                        ALU.subtract)
                uv = wp.tile([128, 4], f32, tag="uv")
                nc.vector.tensor_mul(uv[:], sall[:, 0:4], tgc_a)
                nc.vector.scalar_tensor_tensor(tgc_a[:, 2:4], uv[:, 0:2], 2.0,
                                               uv[:, 2:4], ALU.mult, ALU.add)

                # ---- ACT: tanh(c);  DVE: h = sig(o) * tanh(c) (bf16) ----
                tc_t = wp.tile([128, 2], f32, tag="tc_t")
                nc.scalar.activation(tc_t[:], tgc_a[:, 2:4], TANH)
                nc.vector.tensor_mul(h_a, sall[:, 4:6], tc_t[:])
                # history write (off critical path, on ACT)
                if isinstance(iv, int):
                    nc.scalar.copy(hist_a[:, iv * 2 + 2:iv * 2 + 4], h_a)
                else:
                    nc.scalar.copy(hist_a[:, bass.ds(iv * 2 + 2, 2)], h_a)

            if unroll == T:
                # straight-line: steps 0..T-2 produce h_2..h_T (slot T-1's
                # h comes from step T-2; no dummy step needed)
                for s in range(T - 1):
                    step(s)
            else:
                with tc.For_i(0, T, unroll, staggered_reset=False,
                              hint_engines=(mybir.EngineType.PE,
                                            mybir.EngineType.Activation,
                                            mybir.EngineType.DVE)) as iv:
                    for s in range(unroll):
                        step(iv + s)

            # ---- output stage: y = W_fc @ h_hist + b_fc  -> [23, T] ----
            hv = hist_a.rearrange("p (t two) -> p t two", two=2)
            y_ps = yp.tile([IN_DIM, T], f32, tag="yps")
            for k in range(2):
                nc.tensor.matmul(y_ps[:], wfc_s.ap()[:, k * IN_DIM:(k + 1) * IN_DIM],
                                 hv[:, 0:T, k],
                                 start=(k == 0), stop=(k == 1),
                                 skip_group_check=True)
            nc.scalar.activation(ysb.ap(), y_ps[:], IDT, bias=bfc_s.ap()[:, 0:1])
            nc.sync.dma_start(yt_d.ap(), ysb.ap())

    nc.compile()
    return nc


def kernel(feature, W_ih, W_hh, b_ih, b_hh, W_fc, b_fc, W_hfc, b_hfc):
    from concourse.bass_utils import run_bass_kernel_spmd

    per_core = _host_prep(feature, W_ih, W_hh, b_ih, b_hh, W_fc, b_fc,
                          W_hfc, b_hfc)

    if "nc" not in _CACHE:
        _CACHE["nc"] = build_program(SEQ_LEN, UNROLL)
    nc = _CACHE["nc"]

    import os
    trace = bool(os.environ.get("LSTM_TRACE"))
    tmpdir = os.environ.get("LSTM_TRACE_DIR") or None
    res = run_bass_kernel_spmd(nc, per_core, list(range(BATCH)),
                               trace=trace, tmpdir=tmpdir)
    _CACHE["last_res"] = res
    out = np.empty((BATCH, SEQ_LEN, IN_DIM), np.float32)
    for bb in range(BATCH):
        out[bb] = res.results[bb]["yt"].T
    return out

